# revision 12
# baseline (speedup 1.0000x reference)
"""DeepGO2 (MLP + GATConv + GO-embedding head) on 8 Trainium2 cores.

Sharding: data-parallel over graph nodes. Each core owns 1250 nodes
(padded to 1280 = 10*128). Phase A computes the GAT projections for the
local node shard; an AllGather shares a per-node bf16 "payload" table
(h | el | q | 1); phase B does the edge-softmax aggregation for the
local dst shard with dma_gather + one-hot segment matmuls; phase C is
the [1280, 10240] logits matmul + sigmoid.

Math identities used (all host-precomputable):
  el = (x@fc_w)@attn_l = x@(fc_w@attn_l)        (and er, q likewise)
  logits[n,g] = sigmoid(agg_n[n]@go[g] + s[n] + rad'[g])
    s[n]    = agg_n[n]@hasFunc  (via payload column q = h@hasFunc)
    rad'[g] = |go_rad[g]| + gat_bias@go[g] + gat_bias@hasFunc
  edge softmax needs no max-subtraction: |e| <= ~2 for this data regime,
  exp() is computed unshifted and normalized by z = sum_e w_e.
"""

import os
import sys

for _p in ("/opt/trn_rl_repo", "/root/.axon_site/_ro/trn_rl_repo"):
    if os.path.isdir(_p) and _p not in sys.path:
        sys.path.insert(0, _p)

import numpy as np
import ml_dtypes

# ---------------------------------------------------------------- constants
N, E, IN, H, G, NZ, R = 10000, 320000, 2560, 1024, 10000, 5000, 10
NC = 8            # cores
NPC = 1250        # real nodes per core
NT = 10           # node tiles per core
NPCP = NT * 128   # padded nodes per core (1280)
IN_T = IN // 128  # 20
H_T = H // 128    # 8
PAY = 1152        # payload cols: h(1024) | el | q | one | pad   (2304B, 9*256)
W2C = H + 3       # fc_w | al2 | ar2 | q2
GP = 10240        # padded GO count
CB = 4            # blocks per dma_gather chunk (512 edges)
BF16 = ml_dtypes.bfloat16


# ---------------------------------------------------------------- host prep
def _host_prep(inputs):
    f32 = np.float32
    features = np.asarray(inputs["features"], f32)
    src = np.asarray(inputs["src"]).astype(np.int64)
    dst = np.asarray(inputs["dst"]).astype(np.int64)
    W1 = np.asarray(inputs["W1"], f32)
    b1 = np.asarray(inputs["b1"], f32)
    fc_w = np.asarray(inputs["fc_w"], f32)
    attn_l = np.asarray(inputs["attn_l"], f32)
    attn_r = np.asarray(inputs["attn_r"], f32)
    gat_bias = np.asarray(inputs["gat_bias"], f32)
    go_embed = np.asarray(inputs["go_embed"], f32)
    go_rad = np.asarray(inputs["go_rad"], f32)
    rel_embed = np.asarray(inputs["rel_embed"], f32)

    hf = rel_embed[R]                      # hasFunc row  [H]
    al2 = fc_w @ attn_l                    # [H]
    ar2 = fc_w @ attn_r
    q2 = fc_w @ hf
    w2e = np.concatenate([fc_w, al2[:, None], ar2[:, None], q2[:, None]], axis=1)

    go = go_embed[:G]                      # [G, H]
    goT = np.zeros((H, GP), f32)
    goT[:, :G] = go.T
    radp = np.zeros((1, GP), f32)
    radp[0, :G] = np.abs(go_rad[:G, 0]) + go @ gat_bias + float(gat_bias @ hf)

    b1p = b1.reshape(H_T, 128).T.copy()    # [128, H_T]

    # ---- edges: sort by (core, dst-tile), pad per tile ----
    dstc = dst // NPC
    dloc = dst % NPC
    tl = dloc // 128
    dcol = dloc % 128
    group = dstc * NT + tl                 # [E] in [0, 80)
    order = np.argsort(group, kind="stable")
    g_s = group[order]
    src_s = src[order]
    dcol_s = dcol[order]

    counts = np.bincount(group, minlength=NC * NT).reshape(NC, NT)
    maxcnt = counts.max(axis=0)            # per-tile max over cores
    nblk_t = [int(-(-int(m) // 128 // CB * -1)) for m in maxcnt]  # placeholder
    nblk_t = [max(CB, ((int(m) + 127) // 128 + CB - 1) // CB * CB) for m in maxcnt]
    NBT = int(sum(nblk_t))
    EPC = NBT * 128
    blk_base = np.zeros(NT + 1, np.int64)
    blk_base[1:] = np.cumsum(nblk_t)

    # rank of each sorted edge within its group
    gstart = np.zeros(NC * NT + 1, np.int64)
    gstart[1:] = np.cumsum(np.bincount(group, minlength=NC * NT))
    rank = np.arange(E, dtype=np.int64) - gstart[g_s]

    core_s = g_s // NT
    tile_s = g_s % NT
    slot = blk_base[tile_s] * 128 + rank   # slot within the core's padded edges
    srow = NPCP * (src_s // NPC) + (src_s % NPC)  # padded payload row of src

    gi = np.zeros((NC, EPC), np.int16)
    gi[core_s, slot] = srow.astype(np.int16)
    # per-slot local dst column (-1 for padding slots)
    dstloc = np.full((NC, NBT, 128), -1.0, np.float32)
    dstloc[core_s, slot // 128, slot % 128] = dcol_s
    dstloc = np.ascontiguousarray(dstloc.transpose(0, 2, 1))  # [NC,128,NBT] f32

    # wrap gather indices: idx i -> [i % 16, i // 16], replicated to 128 rows
    gi_w = np.ascontiguousarray(
        np.tile(gi.reshape(NC, EPC // 16, 16).transpose(0, 2, 1), (1, 8, 1))
    )                                       # [NC, 128, EPC//16] int16

    in_maps = []
    for c in range(NC):
        ft = np.zeros((IN, NPCP), f32)
        ft[:, :NPC] = features[c * NPC : (c + 1) * NPC].T
        in_maps.append(
            {
                "featT": ft.astype(BF16),
                "w1": W1.astype(BF16),
                "w2e": w2e.astype(BF16),
                "b1p": b1p,
                "goT": goT.astype(BF16),
                "radp": radp,
                "gidx": gi_w[c],
                "dstloc": dstloc[c],
            }
        )
    return in_maps, nblk_t


# ---------------------------------------------------------------- device code
def build_nc(nblk_t):
    import concourse.bacc as bacc
    import concourse.mybir as mybir
    import concourse.tile as tile
    from concourse import library_config
    from concourse.masks import make_identity
    from concourse.tile_autobufs import add_dep_helper

    dt = mybir.dt
    AF = mybir.ActivationFunctionType
    ALU = mybir.AluOpType

    NBT = int(sum(nblk_t))
    EPC = NBT * 128
    blk_base = [0]
    for nb in nblk_t:
        blk_base.append(blk_base[-1] + nb)

    nc = bacc.Bacc("TRN2", target_bir_lowering=False, debug=False, num_devices=NC)

    featT = nc.dram_tensor("featT", [IN, NPCP], dt.bfloat16, kind="ExternalInput")
    w1 = nc.dram_tensor("w1", [IN, H], dt.bfloat16, kind="ExternalInput")
    w2e = nc.dram_tensor("w2e", [H, W2C], dt.bfloat16, kind="ExternalInput")
    b1p = nc.dram_tensor("b1p", [128, H_T], dt.float32, kind="ExternalInput")
    goT = nc.dram_tensor("goT", [H, GP], dt.bfloat16, kind="ExternalInput")
    radp = nc.dram_tensor("radp", [1, GP], dt.float32, kind="ExternalInput")
    gidx = nc.dram_tensor("gidx", [128, EPC // 16], dt.int16, kind="ExternalInput")
    dstloc = nc.dram_tensor("dstloc", [128, NBT], dt.float32, kind="ExternalInput")
    out = nc.dram_tensor("out", [NPCP, GP], dt.float32, kind="ExternalOutput")

    pay_local = nc.dram_tensor("pay_local", [NPCP, PAY], dt.bfloat16)
    pay_full = nc.dram_tensor(
        "pay_full", [NC * NPCP, PAY], dt.bfloat16, addr_space="Shared"
    )

    with tile.TileContext(nc) as tc:
        lib_inst = nc.gpsimd.load_library(library_config.mlp)

        with (
            tc.tile_pool(name="const", bufs=1) as cp,
            tc.tile_pool(name="paydma", bufs=3) as paypool,
        ):
            ident = cp.tile([128, 128], dt.bfloat16)
            make_identity(nc, ident[:])
            ones1 = cp.tile([1, 128], dt.float32)
            nc.vector.memset(ones1[:], 1.0)
            ones1_bf = cp.tile([1, 128], dt.bfloat16)
            nc.vector.memset(ones1_bf[:], 1.0)
            iota_i = cp.tile([128, 128], dt.int32)
            nc.gpsimd.iota(iota_i[:], pattern=[[1, 128]], base=0, channel_multiplier=0)
            iota_bf = cp.tile([128, 128], dt.bfloat16)
            nc.vector.tensor_copy(iota_bf[:], iota_i[:])
            b1_sb = cp.tile([128, H_T], dt.float32)
            nc.sync.dma_start(b1_sb[:], b1p[:])
            er_sb = cp.tile([128, NT], dt.float32)
            er_bf = cp.tile([128, NT], dt.bfloat16)
            s_sb = cp.tile([128, NT], dt.float32)
            xg_sb = cp.tile([128, NT * H], dt.bfloat16)

            pay_dmas = []

            # ---------------- phase A: xT = relu(W1.T-ish), h_ext ----------
            with tc.tile_pool(name="phA", bufs=1) as ap:
                w1_sb = ap.tile([128, IN_T, H], dt.bfloat16)
                nc.sync.dma_start(
                    w1_sb[:], w1.ap().rearrange("(k p) j -> p k j", p=128)
                )
                ft_sb = ap.tile([128, IN_T, NPCP], dt.bfloat16)
                nc.sync.dma_start(
                    ft_sb[:], featT.ap().rearrange("(k p) n -> p k n", p=128)
                )
                w2_sb = ap.tile([128, H_T, W2C], dt.bfloat16)
                nc.sync.dma_start(
                    w2_sb[:], w2e.ap().rearrange("(k p) j -> p k j", p=128)
                )
                xT_sb = ap.tile([128, H_T * NPCP], dt.bfloat16)

                with tc.tile_pool(name="psX", bufs=6, space="PSUM") as psx:
                    for j in range(H_T):
                        for fo in range(0, NPCP, 512):
                            fl = min(512, NPCP - fo)
                            ps = psx.tile([128, fl], dt.float32, tag="psx")
                            for k in range(IN_T):
                                nc.tensor.matmul(
                                    ps[:],
                                    w1_sb[:, k, j * 128 : (j + 1) * 128],
                                    ft_sb[:, k, fo : fo + fl],
                                    start=(k == 0),
                                    stop=(k == IN_T - 1),
                                )
                            nc.scalar.activation(
                                xT_sb[:, j * NPCP + fo : j * NPCP + fo + fl],
                                ps[:],
                                AF.Relu,
                                bias=b1_sb[:, j : j + 1],
                            )

                with (
                    tc.tile_pool(name="psH", bufs=3, space="PSUM") as psh_p,
                    tc.tile_pool(name="psS", bufs=2, space="PSUM") as pss_p,
                ):
                  for n in range(NT):
                    psh = psh_p.tile([128, H], dt.float32)
                    pss = pss_p.tile([128, 3], dt.float32)
                    for fo in range(0, H, 512):
                        for k in range(H_T):
                            nc.tensor.matmul(
                                psh[:, fo : fo + 512],
                                xT_sb[:, k * NPCP + n * 128 : k * NPCP + (n + 1) * 128],
                                w2_sb[:, k, fo : fo + 512],
                                start=(k == 0),
                                stop=(k == H_T - 1),
                            )
                    for k in range(H_T):
                        nc.tensor.matmul(
                            pss[:],
                            xT_sb[:, k * NPCP + n * 128 : k * NPCP + (n + 1) * 128],
                            w2_sb[:, k, H : H + 3],
                            start=(k == 0),
                            stop=(k == H_T - 1),
                        )
                    pay = paypool.tile([128, PAY], dt.bfloat16)
                    nc.vector.tensor_copy(pay[:, 0:H], psh[:])
                    nc.vector.tensor_copy(pay[:, H : H + 1], pss[:, 0:1])
                    nc.vector.tensor_copy(pay[:, H + 1 : H + 2], pss[:, 2:3])
                    nc.vector.memset(pay[:, H + 2 : H + 3], 1.0)
                    nc.vector.memset(pay[:, H + 3 : PAY], 0.0)
                    nc.vector.tensor_copy(er_sb[:, n : n + 1], pss[:, 1:2])
                    d = nc.sync.dma_start(
                        pay_local[n * 128 : (n + 1) * 128, :], pay[:]
                    )
                    pay_dmas.append(d)
                nc.vector.tensor_copy(er_bf[:], er_sb[:])

            # ---------------- AllGather payload ---------------------------
            cc = nc.gpsimd.collective_compute(
                "AllGather",
                ALU.bypass,
                replica_groups=[list(range(NC))],
                ins=[pay_local[:]],
                outs=[pay_full[:]],
            )
            for d in pay_dmas:
                add_dep_helper(cc.ins, d.ins, sync=True, reason="cc after payload")

            # ---------------- phase B: edge aggregation -------------------
            with (
                tc.tile_pool(name="phB", bufs=1) as bp,
                tc.tile_pool(name="erbc", bufs=2) as ebp,
                tc.tile_pool(name="gat", bufs=3) as gp,
                tc.tile_pool(name="lw", bufs=4) as lwp,
                tc.tile_pool(name="psAgg", bufs=1, space="PSUM") as psagg,
                tc.tile_pool(name="psEr", bufs=2, space="PSUM") as pser,
                tc.tile_pool(name="small", bufs=4) as smp,
            ):
                gidx_sb = bp.tile([128, EPC // 16], dt.int16)
                nc.sync.dma_start(gidx_sb[:], gidx[:])
                dl_sb = bp.tile([128, NBT], dt.float32)
                nc.sync.dma_start(dl_sb[:], dstloc[:])

                for t in range(NT):
                    nbt = nblk_t[t]
                    # er_bc[e, d] = er[tile t][d]  — 2-matmul partition broadcast
                    erp1 = pser.tile([1, 128], dt.float32, tag="erp1")
                    nc.tensor.matmul(erp1[:], er_bf[:, t : t + 1], ident[:])
                    erow = smp.tile([1, 128], dt.bfloat16, tag="erow")
                    nc.vector.tensor_copy(erow[:], erp1[:])
                    erp2 = pser.tile([128, 128], dt.float32, tag="erp2")
                    nc.tensor.matmul(erp2[:], ones1_bf[:], erow[:])
                    er_bc = ebp.tile([128, 128], dt.bfloat16, tag="erbc")
                    nc.vector.tensor_copy(er_bc[:], erp2[:])

                    ps0 = psagg.tile([128, 512], dt.float32, tag="agg0")
                    ps1 = psagg.tile([128, 512], dt.float32, tag="agg1")
                    psz = psagg.tile([128, 3], dt.float32, tag="aggz")

                    for c in range(nbt // CB):
                        gt = gp.tile([128, CB, PAY], dt.bfloat16, tag="gat")
                        icol = (blk_base[t] + c * CB) * 8
                        gd = nc.gpsimd.dma_gather(
                            gt[:],
                            pay_full[:],
                            gidx_sb[:, icol : icol + CB * 8],
                            CB * 128,
                            CB * 128,
                            PAY,
                        )
                        add_dep_helper(gd.ins, lib_inst.ins, sync=False,
                                       reason="gather after lib")
                        add_dep_helper(gd.ins, cc.ins, sync=True,
                                       reason="gather after allgather")
                        for b in range(CB):
                            blk = c * CB + b
                            # es = er_bc + el_src   (el rides in payload col H)
                            elf = lwp.tile([128, 1], dt.float32, tag="elf")
                            nc.vector.tensor_copy(elf[:], gt[:, b, H : H + 1])
                            es = lwp.tile([128, 128], dt.bfloat16, tag="es")
                            nc.vector.tensor_scalar_add(es[:], er_bc[:], elf[:])
                            # lr = leaky_relu(es) = max(0.2*es, es)
                            lr = lwp.tile([128, 128], dt.bfloat16, tag="lr")
                            nc.vector.scalar_tensor_tensor(
                                lr[:], es[:], 0.2, es[:], op0=ALU.mult, op1=ALU.max
                            )
                            # w = exp(lr)
                            wt = lwp.tile([128, 128], dt.bfloat16, tag="wt")
                            nc.scalar.activation(wt[:], lr[:], AF.Exp)
                            # lw = (iota == dstloc) * w
                            lw = lwp.tile([128, 128], dt.bfloat16, tag="lw")
                            nc.vector.scalar_tensor_tensor(
                                lw[:],
                                iota_bf[:],
                                dl_sb[:, blk_base[t] + blk : blk_base[t] + blk + 1],
                                wt[:],
                                op0=ALU.is_equal,
                                op1=ALU.mult,
                            )
                            first = blk == 0
                            last = blk == nbt - 1
                            nc.tensor.matmul(
                                ps0[:], lw[:], gt[:, b, 0:512],
                                start=first, stop=last,
                            )
                            nc.tensor.matmul(
                                ps1[:], lw[:], gt[:, b, 512:1024],
                                start=first, stop=last,
                            )
                            nc.tensor.matmul(
                                psz[:], lw[:], gt[:, b, H : H + 3],
                                start=first, stop=last,
                            )

                    zc = smp.tile([128, 1], dt.float32, tag="zc")
                    nc.vector.tensor_scalar_max(zc[:], psz[:, 2:3], 1e-30)
                    rz = smp.tile([128, 1], dt.float32, tag="rz")
                    nc.vector.reciprocal(rz[:], zc[:])
                    nc.vector.tensor_tensor(
                        s_sb[:, t : t + 1], psz[:, 1:2], rz[:], op=ALU.mult
                    )
                    nc.scalar.mul(xg_sb[:, t * H : t * H + 512], ps0[:], rz[:])
                    nc.scalar.mul(xg_sb[:, t * H + 512 : (t + 1) * H], ps1[:], rz[:])

            # ---------------- phase C: logits ----------------------------
            with (
                tc.tile_pool(name="phC", bufs=1) as cpc,
                tc.tile_pool(name="goTp", bufs=2) as gop,
                tc.tile_pool(name="outp", bufs=4) as outp,
            ):
                rad_sb = cpc.tile([1, GP], dt.float32)
                nc.sync.dma_start(rad_sb[:], radp[:])
                xgT_sb = cpc.tile([128, H_T * NPCP], dt.bfloat16)
                with tc.tile_pool(name="psT", bufs=4, space="PSUM") as pst_p:
                    for t in range(NT):
                        for k in range(H_T):
                            pst = pst_p.tile([128, 128], dt.bfloat16, tag="pst")
                            nc.tensor.transpose(
                                pst[:],
                                xg_sb[:, t * H + k * 128 : t * H + (k + 1) * 128],
                                ident[:],
                            )
                            nc.vector.tensor_copy(
                                xgT_sb[
                                    :, k * NPCP + t * 128 : k * NPCP + (t + 1) * 128
                                ],
                                pst[:],
                            )
                with tc.tile_pool(name="psC", bufs=8, space="PSUM") as psc_p:
                  GB = 2048  # g columns per goT staging block
                  for gb in range(GP // GB):
                    goT_sb = gop.tile([128, H_T, GB], dt.bfloat16, tag="goT")
                    nc.sync.dma_start(
                        goT_sb[:],
                        goT.ap()[:, gb * GB : (gb + 1) * GB].rearrange(
                            "(k p) g -> p k g", p=128
                        ),
                    )
                    for n in range(NT):
                        pss = []
                        for gc in range(GB // 512):
                            ps = psc_p.tile([128, 512], dt.float32, tag="psc")
                            g0 = gb * GB + gc * 512
                            nc.tensor.matmul(
                                ps[:],
                                ones1[:],
                                rad_sb[:, g0 : g0 + 512],
                                start=True,
                                stop=False,
                            )
                            pss.append(ps)
                        for k in range(H_T):
                            for gc in range(GB // 512):
                                nc.tensor.matmul(
                                    pss[gc][:],
                                    xgT_sb[
                                        :, k * NPCP + n * 128 : k * NPCP + (n + 1) * 128
                                    ],
                                    goT_sb[:, k, gc * 512 : (gc + 1) * 512],
                                    start=False,
                                    stop=(k == H_T - 1),
                                )
                        for gc in range(GB // 512):
                            g0 = gb * GB + gc * 512
                            ot = outp.tile([128, 512], dt.float32, tag="ot")
                            nc.scalar.activation(
                                ot[:], pss[gc][:], AF.Sigmoid, bias=s_sb[:, n : n + 1]
                            )
                            nc.sync.dma_start(
                                out[n * 128 : (n + 1) * 128, g0 : g0 + 512], ot[:]
                            )

    nc.compile()
    return nc


# ---------------------------------------------------------------- entry point
def kernel(**inputs):
    from concourse.bass_utils import run_bass_kernel_spmd

    in_maps, nblk_t = _host_prep(inputs)
    nc = build_nc(nblk_t)
    res = run_bass_kernel_spmd(nc, in_maps, list(range(NC)))
    full = np.empty((N, G), np.float32)
    for c in range(NC):
        full[c * NPC : (c + 1) * NPC] = res.results[c]["out"][:NPC, :G]
    return full


if __name__ == "__main__":
    # quick self-run with random data (no reference check)
    rng = np.random.default_rng(0)
    ins = {
        "features": rng.standard_normal((N, IN), np.float32),
        "src": rng.integers(0, N, E),
        "dst": rng.integers(0, N, E),
        "W1": rng.standard_normal((IN, H), np.float32) * 0.02,
        "b1": np.zeros(H, np.float32),
        "fc_w": rng.standard_normal((H, H), np.float32) * 0.02,
        "attn_l": rng.standard_normal(H, np.float32) * 0.02,
        "attn_r": rng.standard_normal(H, np.float32) * 0.02,
        "gat_bias": np.zeros(H, np.float32),
        "go_embed": rng.standard_normal((G + NZ, H), np.float32) * 0.02,
        "go_rad": rng.standard_normal((G + NZ, 1), np.float32) * 0.02,
        "rel_embed": rng.standard_normal((R + 1, H), np.float32) * 0.02,
    }
    out = kernel(**ins)
    print("out", out.shape, out.dtype, out[:2, :4])
